# revision 25
# baseline (speedup 1.0000x reference)
"""MultiLabelContrastiveLoss Trainium2 kernel (8 NeuronCores, Bass/Tile).

Math (reference):
    sim = (emb @ emb.T) / T                      # [B, B]
    cnt[i,j] = #aspects where labels match       # via one-hot GEMM
    positive_mask = (cnt/A >= 0.5) & offdiag
    pos_i = sum_j exp(sim) * positive_mask
    all_i = sum_j exp(sim) * offdiag
    valid = pos > 0
    loss = sum(valid * -log(where(valid,pos,1)/(all+eps))) / max(n_valid, 1)

Kernel strategy (SPMD over 8 cores): exploit sim/mask SYMMETRY — compute
each unordered block-pair once, harvesting BOTH sides:
  - rows: free-axis accumulation (ACT accum for `all`, DVE STT accum for
    `pos`),
  - cols: PE ones-matmul column sums of the E / masked-E (junk) tiles.
Per core (own row-block r = core id c, local column space of 5120 cols):
  L0 = block c  (full 1024, diag excluded via -1e30*I PSUM accumulate,
                 row sums only — both orders inside the block)
  L1..L3 = blocks c+1..c+3 (full, row sums + col sums)
  L4 = block c+4: quadrant split with c+4's core — row-tiles 0-3 process
       local cols [0,512), 4-7 [512,1024); host rotates block data by 512
       for cores >= 4 so the two cores cover complementary quadrants.
Work: 36 eq-tiles/core vs 64 for the row-stripe baseline.
Partial row/col sums are scattered (indirect DMA, per-core index tensors)
into a global-row-indexed [128,128] DRAM buffer, ReduceScatter'd across
the 8 cores (chunk c = rows of block c), and the log-epilogue runs on
device per core -> [1,2] (contrib_sum, n_valid); host combines 8 pairs.
"""

import numpy as np
import ml_dtypes

import concourse.tile as tile
from concourse import bacc, bass, mybir
from concourse.bass_utils import run_bass_kernel_spmd

F32 = mybir.dt.float32
BF16 = mybir.dt.bfloat16
I32 = mybir.dt.int32

B = 8192          # batch
D = 128           # embedding dim
A = 12            # aspects
NCLS = 3          # classes
TEMP = 0.07
EPS = 1e-8
NEG = -1.0e30
N_CORES = 8
BLK = B // N_CORES            # 1024 rows per block
W = 5 * BLK                   # local column space (L4 uses 512/row-tile)
# one-hot GEMM: K=37 (12 aspects x 3 classes + augment), duplicated at
# partitions 0 and 64 so the two 512-wide cnt matmuls of a tile run
# concurrently in distinct PE row-groups.
KOH = 37
NST = 5                       # super-tiles per core
NSLOT = 2 * 8 * NST           # pos accum slots (2 cnt halves per tile)


def build(n_cores=N_CORES, debug_outputs=False):
    nc = bacc.Bacc("TRN2", target_bir_lowering=False, debug=False,
                   num_devices=n_cores)
    embT_d = nc.dram_tensor("embT", [D, W], BF16, kind="ExternalInput")
    labT_d = nc.dram_tensor("labT", [KOH, W], BF16, kind="ExternalInput")
    aug_d = nc.dram_tensor("aug", [2, W], BF16, kind="ExternalInput")
    ident_d = nc.dram_tensor("identb", [128, 128], BF16, kind="ExternalInput")
    dneg_d = nc.dram_tensor("diagnegb", [128, 128], BF16, kind="ExternalInput")
    # per-core partial sums; the cross-core reduce + tiny log epilogue is
    # part of the host-side unshard (device collectives cost 40-80us fixed
    # under this runtime -- measured, unusable at this kernel size)
    rowpart_d = nc.dram_tensor("rowpart", [128, 16], F32,
                               kind="ExternalOutput")
    colpart_d = nc.dram_tensor("colpart", [8, BLK], F32,
                               kind="ExternalOutput")

    with tile.TileContext(nc) as tc:
        with (
            tc.tile_pool(name="const", bufs=1) as cpool,
            tc.tile_pool(name="ework", bufs=12) as epool,
            tc.tile_pool(name="junk", bufs=12) as jpool,
            tc.tile_pool(name="psA", bufs=2, space="PSUM") as psA,
            tc.tile_pool(name="psB", bufs=2, space="PSUM") as psB,
            tc.tile_pool(name="psC", bufs=1, space="PSUM") as psC,
        ):
            emb_sb = cpool.tile([D, W], BF16)
            lab_sb = cpool.tile([64 + KOH, W], BF16)
            ident_sb = cpool.tile([128, 128], BF16)
            dneg_sb = cpool.tile([128, 128], BF16)
            ohL = cpool.tile([64 + KOH, BLK], BF16)
            ohR = cpool.tile([64 + KOH, W], BF16)
            ones_sb = cpool.tile([128, 1], BF16)
            pos_acc = cpool.tile([128, NSLOT], F32)
            all_acc = cpool.tile([128, 8 * NST], F32)

            # ---- input DMAs, spread over the per-engine DMA queues so the
            # first tiles' operands land fast ----
            nc.sync.dma_start(emb_sb[:, 0:512], embT_d[:, 0:512])
            nc.gpsimd.dma_start(emb_sb[:, 512:1024], embT_d[:, 512:1024])
            nc.sync.dma_start(ident_sb[:], ident_d[:])
            nc.sync.dma_start(dneg_sb[:], dneg_d[:])
            nc.scalar.dma_start(lab_sb[0:KOH, 0:BLK], labT_d[:, 0:BLK])
            nc.scalar.dma_start(lab_sb[64:64 + KOH, 0:BLK], labT_d[:, 0:BLK])
            nc.scalar.dma_start(ohR[36:37, :], aug_d[0:1, :])
            nc.scalar.dma_start(ohR[100:101, :], aug_d[0:1, :])
            nc.scalar.dma_start(ohL[36:37, :], aug_d[1:2, 0:BLK])
            nc.scalar.dma_start(ohL[100:101, :], aug_d[1:2, 0:BLK])
            qs = [nc.sync, nc.gpsimd]
            for k in range(1, W // 1024):
                sl = slice(k * 1024, (k + 1) * 1024)
                qs[(k - 1) % 2].dma_start(emb_sb[:, sl], embT_d[:, sl])
            nc.scalar.dma_start(lab_sb[0:KOH, BLK:W], labT_d[:, BLK:W])
            nc.scalar.dma_start(lab_sb[64:64 + KOH, BLK:W], labT_d[:, BLK:W])
            # ---- one-hot build (bf16, 4x mode on DVE) ----
            nc.vector.tensor_scalar(
                out=ohR[0:36, 0:BLK], in0=lab_sb[0:36, 0:BLK],
                scalar1=0.0, scalar2=None, op0=mybir.AluOpType.is_equal)
            nc.vector.tensor_scalar(
                out=ohR[64:100, 0:BLK], in0=lab_sb[64:100, 0:BLK],
                scalar1=0.0, scalar2=None, op0=mybir.AluOpType.is_equal)
            nc.vector.tensor_scalar(
                out=ohL[0:36, :], in0=lab_sb[0:36, 0:BLK],
                scalar1=0.0, scalar2=None, op0=mybir.AluOpType.is_equal)
            nc.vector.tensor_scalar(
                out=ohL[64:100, :], in0=lab_sb[64:100, 0:BLK],
                scalar1=0.0, scalar2=None, op0=mybir.AluOpType.is_equal)
            nc.vector.tensor_scalar(
                out=ohR[0:36, BLK:W], in0=lab_sb[0:36, BLK:W],
                scalar1=0.0, scalar2=None, op0=mybir.AluOpType.is_equal)
            nc.vector.tensor_scalar(
                out=ohR[64:100, BLK:W], in0=lab_sb[64:100, BLK:W],
                scalar1=0.0, scalar2=None, op0=mybir.AluOpType.is_equal)

            nc.vector.memset(ones_sb[:], 1.0)
            nc.vector.memset(pos_acc[:], 0.0)

            # E / junk tiles per super-tile (kept for the colsum bursts)
            etiles = [[None] * 8 for _ in range(NST)]
            jtiles = [[None] * 8 for _ in range(NST)]

            def tile_cols(st, rl):
                """Local column range of tile (st, rl) and its width."""
                if st < 4:
                    return st * 1024, 1024
                return 4 * 1024 + (0 if rl < 4 else 512), 512

            def emit_mask_stage(pend):
                """cnt GEMM halves + masked STT for a finished sim tile."""
                st, rl, e_t = pend
                c0, w = tile_cols(st, rl)
                rsl = slice(128 * rl, 128 * rl + 128)
                junk = jpool.tile([128, 1024], BF16, tag="junk",
                                  name=f"junk{st}_{rl}")
                jtiles[st][rl] = junk
                cps = []
                for h in range(w // 512):
                    # the two halves use the one-hot copies at partitions 0
                    # and 64 -> distinct PE row-groups, concurrent matmuls
                    cnt_ps = psB.tile([128, 512], F32, tag="cnt",
                                      name=f"cnt{st}_{rl}_{h}")
                    csl = slice(c0 + 512 * h, c0 + 512 * (h + 1))
                    kp = slice(64 * h, 64 * h + KOH)
                    nc.tensor.matmul(cnt_ps[:], ohL[kp, rsl], ohR[kp, csl],
                                     start=True, stop=True,
                                     tile_position=(64 * h, 0))
                    cps.append(cnt_ps)
                for h, cnt_ps in enumerate(cps):
                    osl = slice(512 * h, 512 * (h + 1))
                    slot = (rl * NST + st) * 2 + h
                    nc.vector.scalar_tensor_tensor(
                        out=junk[:, osl],
                        in0=cnt_ps[:], scalar=0.0, in1=e_t[:, osl],
                        op0=mybir.AluOpType.is_ge,
                        op1=mybir.AluOpType.mult,
                        accum_out=pos_acc[:, slot:slot + 1],
                    )

            def emit_colsum_burst(st):
                """Column sums of E and junk tiles of super-tile st (>=1)
                via ones-matmuls, PE-accumulated, then DMA'd to staging."""
                colps = psC.tile([33, 1024], F32, tag="col", name=f"col{st}")
                for rl in range(8):
                    c0, w = tile_cols(st, rl)
                    if st == 4:
                        start, stop = rl % 4 == 0, rl % 4 == 3
                    else:
                        start, stop = rl == 0, rl == 7
                    for hh in range(w // 512):
                        osl = slice((c0 - 4096) if st == 4 else 512 * hh,
                                    ((c0 - 4096) if st == 4 else 512 * hh) + 512)
                        isl = slice(512 * hh, 512 * hh + 512)
                        for v, tiles in enumerate((jtiles, etiles)):
                            vp = 32 * v
                            nc.tensor.matmul(
                                colps[vp:vp + 1, osl], ones_sb[:],
                                tiles[st][rl][:, isl],
                                start=start, stop=stop,
                                tile_position=(0, vp),
                            )
                # PSUM is not DMA-readable: evacuate via one [33,1024] copy
                # (covers both vectors' partitions), alternating engines.
                colsb = cpool.tile([33, 1024], F32, name=f"colsb{st}")
                if st % 2 == 0:
                    nc.vector.tensor_copy(colsb[:], colps[:])
                else:
                    nc.scalar.copy(colsb[:], colps[:])
                for v in range(2):
                    nc.sync.dma_start(
                        colpart_d[2 * (st - 1) + v:2 * (st - 1) + v + 1, :],
                        colsb[32 * v:32 * v + 1, :])

            # ---- main loop (software-pipelined as in the row-stripe
            # baseline: mask stage of tile k-1 emitted after sim+exp of
            # tile k; colsum burst of super-tile st-1 emitted inside
            # (st, rl==2)) ----
            pending = None
            for st in range(NST):
                for rl in range(8):
                    c0, w = tile_cols(st, rl)
                    rsl = slice(128 * rl, 128 * rl + 128)
                    sim_ps = psA.tile([128, 1024], F32, tag="sim",
                                      name=f"sim{st}_{rl}")
                    dloc = 128 * rl  # diag offset within L0 (local cols)
                    for h in range(w // 512):
                        csl = slice(c0 + 512 * h, c0 + 512 * (h + 1))
                        osl = slice(512 * h, 512 * (h + 1))
                        dh = st == 0 and 512 * h <= dloc < 512 * (h + 1)
                        nc.tensor.matmul(sim_ps[:, osl], emb_sb[:, rsl],
                                         emb_sb[:, csl], start=True,
                                         stop=not dh)
                        if dh:
                            dsl = slice(dloc, dloc + 128)
                            nc.tensor.matmul(sim_ps[:, dsl], ident_sb[:],
                                             dneg_sb[:], start=False,
                                             stop=True)
                    e_t = epool.tile([128, 1024], BF16, tag="E",
                                     name=f"E{st}_{rl}")
                    etiles[st][rl] = e_t
                    aslot = rl * NST + st
                    nc.scalar.activation(
                        e_t[:, 0:w], sim_ps[:, 0:w],
                        mybir.ActivationFunctionType.Exp,
                        scale=1.0 / TEMP,
                        accum_out=all_acc[:, aslot:aslot + 1],
                    )
                    if pending is not None:
                        emit_mask_stage(pending)
                    pending = (st, rl, e_t)
                    if rl == 2 and st >= 2:
                        emit_colsum_burst(st - 1)
            emit_mask_stage(pending)
            emit_colsum_burst(4)

            # ---- row-partial reduction -> [128, 16] out ----
            pr_pack = cpool.tile([128, 16], F32)
            nc.vector.reduce_sum(
                pr_pack[:, 0:8],
                pos_acc[:].rearrange("p (r q) -> p r q", q=2 * NST),
                axis=mybir.AxisListType.X)
            nc.vector.reduce_sum(
                pr_pack[:, 8:16],
                all_acc[:].rearrange("p (r q) -> p r q", q=NST),
                axis=mybir.AxisListType.X)
            nc.sync.dma_start(rowpart_d[:], pr_pack[:])

    nc.compile()
    return nc


_CACHE = {}


def _get_nc():
    if "nc" not in _CACHE:
        _CACHE["nc"] = build()
    return _CACHE["nc"]


def _local_cols(c):
    """Global column indices of core c's local [0, W) column space."""
    cols = []
    for s in range(4):
        b = (c + s) % N_CORES
        cols.extend(range(b * BLK, (b + 1) * BLK))
    b4 = (c + 4) % N_CORES
    half = np.arange(BLK)
    if c >= 4:
        half = np.roll(half, -512)
    cols.extend(b4 * BLK + half)
    return np.array(cols)


def make_in_maps(embeddings, labels):
    embT = np.ascontiguousarray(
        embeddings.astype(np.float32).T).astype(ml_dtypes.bfloat16)  # [D, B]
    labT = np.full((KOH, B), -1.0, dtype=np.float32)
    lt = np.asarray(labels).astype(np.float32)          # [B, A]
    for a in range(A):
        for c in range(NCLS):
            labT[3 * a + c] = lt[:, a] - c
    labT = labT.astype(ml_dtypes.bfloat16)
    ident = np.eye(128, dtype=np.float32).astype(ml_dtypes.bfloat16)
    dneg = (np.eye(128, dtype=np.float32) * NEG).astype(ml_dtypes.bfloat16)
    aug = np.empty((2, W), dtype=np.float32)
    aug[0] = -(A / 2.0 - 0.5)
    aug[1] = 1.0
    aug = aug.astype(ml_dtypes.bfloat16)

    in_maps = []
    for c in range(N_CORES):
        cols = _local_cols(c)
        in_maps.append({
            "embT": np.ascontiguousarray(embT[:, cols]),
            "labT": np.ascontiguousarray(labT[:, cols]),
            "aug": aug, "identb": ident, "diagnegb": dneg,
        })
    return in_maps


def combine_partials(results):
    """Cross-core reduce of the partial row/col sums + scalar epilogue
    (the unshard step; O(B) host work)."""
    pos = np.zeros(B, dtype=np.float64)
    alls = np.zeros(B, dtype=np.float64)
    for c in range(N_CORES):
        rp = results[c]["rowpart"].astype(np.float64)     # [128, 16]
        cp = results[c]["colpart"].astype(np.float64)     # [8, BLK]
        rows = c * BLK + (np.arange(8) * 128)[:, None] + np.arange(128)
        np.add.at(pos, rows.reshape(-1), rp[:, 0:8].T.reshape(-1))
        np.add.at(alls, rows.reshape(-1), rp[:, 8:16].T.reshape(-1))
        cols = _local_cols(c)
        for st in range(1, 5):
            idx = cols[st * BLK:(st + 1) * BLK]
            pos[idx] += cp[2 * (st - 1)]
            alls[idx] += cp[2 * (st - 1) + 1]
    valid = pos > 0
    per = np.log(alls + EPS) - np.log(np.where(valid, pos, 1.0))
    nv = int(valid.sum())
    loss = np.where(valid, per, 0.0).sum() / max(nv, 1) if nv > 0 else 0.0
    return np.array(loss, dtype=np.float32)


def kernel(embeddings, labels):
    nc = _get_nc()
    in_maps = make_in_maps(embeddings, labels)
    res = run_bass_kernel_spmd(nc, in_maps, core_ids=list(range(N_CORES)))
    return combine_partials(res.results)


# revision 26
# speedup vs baseline: 1.5290x; 1.5290x over previous
"""MultiLabelContrastiveLoss Trainium2 kernel (8 NeuronCores, Bass/Tile).

Math (reference):
    sim = (emb @ emb.T) / T                      # [B, B]
    cnt[i,j] = #aspects where labels match       # via one-hot GEMM
    positive_mask = (cnt/A >= 0.5) & offdiag
    pos_i = sum_j exp(sim) * positive_mask
    all_i = sum_j exp(sim) * offdiag
    valid = pos > 0
    loss = sum(valid * -log(where(valid,pos,1)/(all+eps))) / max(n_valid, 1)

Kernel strategy (SPMD over 8 cores): exploit sim/mask SYMMETRY — compute
each unordered block-pair once, harvesting BOTH sides of every pair:
  - row side: free-axis accumulation (ACT accum_out for `all`, DVE
    masked-STT accum_out for `pos`),
  - col side: PE ones-matmul column sums of the E and masked-E (junk)
    tiles (accumulated in PSUM across the super-tile, two vectors
    col-packed at output partitions 0/32).
Per core (own row-block = core id c; local column space of 5120 cols):
  L0 = block c  (full 1024, diag excluded via -1e30*I PSUM accumulate,
                 row sums only — both pair orders are inside the block)
  L1..L3 = blocks c+1..c+3 (full, row sums + col sums)
  L4 = block c+4: quadrant split against core c+4 — row-tiles 0-3
       process local cols [0,512), 4-7 [512,1024); the host rotates the
       block's columns by 512 for cores >= 4 so the two cores cover
       complementary quadrants of the pair block.
Work: 36 eq-tiles/core vs 64 for the row-stripe baseline.
The cross-core reduction of the per-row partial sums plus the O(B) log
epilogue happens in the host-side combine (unshard) step: on-device
collectives cost 40-80us fixed under this runtime (measured via
ReduceScatter/AllGather microbenchmarks) — unusable at this kernel size.
"""

import numpy as np
import ml_dtypes

import concourse.tile as tile
from concourse import bacc, mybir
from concourse.bass_utils import run_bass_kernel_spmd

F32 = mybir.dt.float32
BF16 = mybir.dt.bfloat16

B = 8192          # batch
D = 128           # embedding dim
A = 12            # aspects
NCLS = 3          # classes
TEMP = 0.07
EPS = 1e-8
NEG = -1.0e30
N_CORES = 8
BLK = B // N_CORES            # 1024 rows per block
W = 5 * BLK                   # local column space (L4 uses 512/row-tile)
# one-hot GEMM layout: class c's 12 rows live at partitions [32c, 32c+12)
# (engine write base partitions must be 32-aligned), augment row at
# partition 96, gap rows zeroed -> contraction K = 97.
KOH = 97
NST = 5                       # super-tiles per core
NSLOT = 2 * 8 * NST           # pos accum slots (2 cnt halves per tile)


def build(n_cores=N_CORES):
    nc = bacc.Bacc("TRN2", target_bir_lowering=False, debug=False,
                   num_devices=n_cores)
    embT_d = nc.dram_tensor("embT", [D, W], BF16, kind="ExternalInput")
    # labels pre-shifted per class on host: row 32c+a = label[a]-c, filler -1
    labT_d = nc.dram_tensor("labT", [96, W], BF16, kind="ExternalInput")
    # augment rows (row0: -5.5 for ohR, row1: 1.0 for ohL)
    aug_d = nc.dram_tensor("aug", [2, W], BF16, kind="ExternalInput")
    ident_d = nc.dram_tensor("identb", [128, 128], BF16, kind="ExternalInput")
    dneg_d = nc.dram_tensor("diagnegb", [128, 128], BF16, kind="ExternalInput")
    rowpart_d = nc.dram_tensor("rowpart", [128, 16], F32,
                               kind="ExternalOutput")
    colpart_d = nc.dram_tensor("colpart", [8, BLK], F32,
                               kind="ExternalOutput")

    with tile.TileContext(nc) as tc:
        with (
            tc.tile_pool(name="const", bufs=1) as cpool,
            tc.tile_pool(name="ework", bufs=12) as epool,
            tc.tile_pool(name="junk", bufs=12) as jpool,
            tc.tile_pool(name="psA", bufs=2, space="PSUM") as psA,
            tc.tile_pool(name="psB", bufs=2, space="PSUM") as psB,
            tc.tile_pool(name="psC", bufs=1, space="PSUM") as psC,
        ):
            emb_sb = cpool.tile([D, W], BF16)
            lab_sb = cpool.tile([96, W], BF16)
            ident_sb = cpool.tile([128, 128], BF16)
            dneg_sb = cpool.tile([128, 128], BF16)
            ohL = cpool.tile([KOH, BLK], BF16)
            ohR = cpool.tile([KOH, W], BF16)
            ones_sb = cpool.tile([128, 1], BF16)
            pos_acc = cpool.tile([128, NSLOT], F32)
            all_acc = cpool.tile([128, 8 * NST], F32)

            # ---- input DMAs, spread across the DMA-capable queues so the
            # first tiles' operands land fast ----
            nc.sync.dma_start(emb_sb[:, 0:512], embT_d[:, 0:512])
            nc.gpsimd.dma_start(emb_sb[:, 512:1024], embT_d[:, 512:1024])
            nc.sync.dma_start(ident_sb[:], ident_d[:])
            nc.sync.dma_start(dneg_sb[:], dneg_d[:])
            nc.scalar.dma_start(lab_sb[:, 0:BLK], labT_d[:, 0:BLK])
            nc.scalar.dma_start(ohR[96:97, :], aug_d[0:1, :])
            nc.scalar.dma_start(ohL[96:97, :], aug_d[1:2, 0:BLK])
            qs = [nc.sync, nc.gpsimd]
            for k in range(1, W // 1024):
                sl = slice(k * 1024, (k + 1) * 1024)
                qs[(k - 1) % 2].dma_start(emb_sb[:, sl], embT_d[:, sl])
            nc.scalar.dma_start(lab_sb[:, BLK:W], labT_d[:, BLK:W])

            # ---- one-hot build (bf16, 4x mode on DVE) ----
            nc.vector.tensor_scalar(
                out=ohR[0:96, 0:BLK], in0=lab_sb[:, 0:BLK],
                scalar1=0.0, scalar2=None, op0=mybir.AluOpType.is_equal)
            nc.vector.tensor_scalar(
                out=ohL[0:96, :], in0=lab_sb[:, 0:BLK],
                scalar1=0.0, scalar2=None, op0=mybir.AluOpType.is_equal)
            nc.vector.tensor_scalar(
                out=ohR[0:96, BLK:W], in0=lab_sb[:, BLK:W],
                scalar1=0.0, scalar2=None, op0=mybir.AluOpType.is_equal)

            nc.vector.memset(ones_sb[:], 1.0)
            nc.vector.memset(pos_acc[:], 0.0)

            # E / junk tiles per super-tile (kept for the colsum bursts)
            etiles = [[None] * 8 for _ in range(NST)]
            jtiles = [[None] * 8 for _ in range(NST)]

            def tile_cols(st, rl):
                """Local column range of tile (st, rl) and its width."""
                if st < 4:
                    return st * 1024, 1024
                return 4 * 1024 + (0 if rl < 4 else 512), 512

            def emit_mask_stage(pend):
                """cnt GEMM halves + masked STT for a finished sim tile."""
                st, rl, e_t = pend
                c0, w = tile_cols(st, rl)
                rsl = slice(128 * rl, 128 * rl + 128)
                junk = jpool.tile([128, 1024], BF16, tag="junk",
                                  name=f"junk{st}_{rl}")
                jtiles[st][rl] = junk
                for h in range(w // 512):
                    cnt_ps = psB.tile([128, 512], F32, tag="cnt",
                                      name=f"cnt{st}_{rl}_{h}")
                    csl = slice(c0 + 512 * h, c0 + 512 * (h + 1))
                    osl = slice(512 * h, 512 * (h + 1))
                    nc.tensor.matmul(cnt_ps[:], ohL[:, rsl], ohR[:, csl],
                                     start=True, stop=True)
                    slot = (rl * NST + st) * 2 + h
                    nc.vector.scalar_tensor_tensor(
                        out=junk[:, osl],
                        in0=cnt_ps[:], scalar=0.0, in1=e_t[:, osl],
                        op0=mybir.AluOpType.is_ge,
                        op1=mybir.AluOpType.mult,
                        accum_out=pos_acc[:, slot:slot + 1],
                    )

            def emit_colsum_burst(st):
                """Column sums of junk (pos, psum partition 0) and E (all,
                partition 32) tiles of super-tile st >= 1: ones-matmuls
                PE-accumulated across the 8 row-tiles, then evacuated."""
                colps = psC.tile([33, 1024], F32, tag="col", name=f"col{st}")
                for rl in range(8):
                    c0, w = tile_cols(st, rl)
                    if st == 4:
                        start, stop = rl % 4 == 0, rl % 4 == 3
                    else:
                        start, stop = rl == 0, rl == 7
                    for hh in range(w // 512):
                        o0 = (c0 - 4096) if st == 4 else 512 * hh
                        osl = slice(o0, o0 + 512)
                        isl = slice(512 * hh, 512 * hh + 512)
                        for v, tiles in enumerate((jtiles, etiles)):
                            vp = 32 * v
                            nc.tensor.matmul(
                                colps[vp:vp + 1, osl], ones_sb[:],
                                tiles[st][rl][:, isl],
                                start=start, stop=stop,
                                tile_position=(0, vp),
                            )
                # PSUM is not DMA-readable: one [33,1024] copy (covers both
                # vectors' partitions) on ACT, which has more slack than DVE.
                colsb = cpool.tile([33, 1024], F32, name=f"colsb{st}")
                nc.scalar.copy(colsb[:], colps[:])
                for v in range(2):
                    nc.sync.dma_start(
                        colpart_d[2 * (st - 1) + v:2 * (st - 1) + v + 1, :],
                        colsb[32 * v:32 * v + 1, :])

            # ---- main loop (software-pipelined: the mask stage of tile
            # k-1 is emitted after sim+exp of tile k so the PE's in-order
            # queue never makes ACT wait behind DVE-gated cnt matmuls;
            # colsum burst of super-tile st-1 is emitted inside (st, rl==2))
            pending = None
            for st in range(NST):
                for rl in range(8):
                    c0, w = tile_cols(st, rl)
                    rsl = slice(128 * rl, 128 * rl + 128)
                    sim_ps = psA.tile([128, 1024], F32, tag="sim",
                                      name=f"sim{st}_{rl}")
                    dloc = 128 * rl  # diag offset within L0 (local cols)
                    for h in range(w // 512):
                        csl = slice(c0 + 512 * h, c0 + 512 * (h + 1))
                        osl = slice(512 * h, 512 * (h + 1))
                        dh = st == 0 and 512 * h <= dloc < 512 * (h + 1)
                        nc.tensor.matmul(sim_ps[:, osl], emb_sb[:, rsl],
                                         emb_sb[:, csl], start=True,
                                         stop=not dh)
                        if dh:
                            # exact diag exclusion: accumulate -1e30*I
                            dsl = slice(dloc, dloc + 128)
                            nc.tensor.matmul(sim_ps[:, dsl], ident_sb[:],
                                             dneg_sb[:], start=False,
                                             stop=True)
                    e_t = epool.tile([128, 1024], BF16, tag="E",
                                     name=f"E{st}_{rl}")
                    etiles[st][rl] = e_t
                    aslot = rl * NST + st
                    nc.scalar.activation(
                        e_t[:, 0:w], sim_ps[:, 0:w],
                        mybir.ActivationFunctionType.Exp,
                        scale=1.0 / TEMP,
                        accum_out=all_acc[:, aslot:aslot + 1],
                    )
                    if pending is not None:
                        emit_mask_stage(pending)
                    pending = (st, rl, e_t)
                    if rl == 2 and st >= 2:
                        emit_colsum_burst(st - 1)
            emit_mask_stage(pending)
            emit_colsum_burst(4)

            # ---- row-partial reduction -> [128, 16] out ----
            pr_pack = cpool.tile([128, 16], F32)
            nc.vector.reduce_sum(
                pr_pack[:, 0:8],
                pos_acc[:].rearrange("p (r q) -> p r q", q=2 * NST),
                axis=mybir.AxisListType.X)
            nc.vector.reduce_sum(
                pr_pack[:, 8:16],
                all_acc[:].rearrange("p (r q) -> p r q", q=NST),
                axis=mybir.AxisListType.X)
            nc.sync.dma_start(rowpart_d[:], pr_pack[:])

    nc.compile()
    return nc


_CACHE = {}


def _get_nc():
    if "nc" not in _CACHE:
        _CACHE["nc"] = build()
    return _CACHE["nc"]


def _local_cols(c):
    """Global column indices of core c's local [0, W) column space."""
    cols = []
    for s in range(4):
        b = (c + s) % N_CORES
        cols.extend(range(b * BLK, (b + 1) * BLK))
    b4 = (c + 4) % N_CORES
    half = np.arange(BLK)
    if c >= 4:
        half = np.roll(half, -512)
    cols.extend(b4 * BLK + half)
    return np.array(cols)


def make_in_maps(embeddings, labels):
    embT = np.ascontiguousarray(
        embeddings.astype(np.float32).T).astype(ml_dtypes.bfloat16)  # [D, B]
    labT = np.full((96, B), -1.0, dtype=np.float32)
    lt = np.asarray(labels).astype(np.float32).T
    for c in range(NCLS):
        labT[32 * c:32 * c + A] = lt - c
    labT = labT.astype(ml_dtypes.bfloat16)
    ident = np.eye(128, dtype=np.float32).astype(ml_dtypes.bfloat16)
    dneg = (np.eye(128, dtype=np.float32) * NEG).astype(ml_dtypes.bfloat16)
    aug = np.empty((2, W), dtype=np.float32)
    aug[0] = -(A / 2.0 - 0.5)
    aug[1] = 1.0
    aug = aug.astype(ml_dtypes.bfloat16)

    in_maps = []
    for c in range(N_CORES):
        cols = _local_cols(c)
        in_maps.append({
            "embT": np.ascontiguousarray(embT[:, cols]),
            "labT": np.ascontiguousarray(labT[:, cols]),
            "aug": aug, "identb": ident, "diagnegb": dneg,
        })
    return in_maps


def combine_partials(results):
    """Cross-core reduce of the partial row/col sums + scalar epilogue
    (the unshard step; O(B) host work)."""
    pos = np.zeros(B, dtype=np.float64)
    alls = np.zeros(B, dtype=np.float64)
    for c in range(N_CORES):
        rp = results[c]["rowpart"].astype(np.float64)     # [128, 16]
        cp = results[c]["colpart"].astype(np.float64)     # [8, BLK]
        rows = c * BLK + (np.arange(8) * 128)[:, None] + np.arange(128)
        np.add.at(pos, rows.reshape(-1), rp[:, 0:8].T.reshape(-1))
        np.add.at(alls, rows.reshape(-1), rp[:, 8:16].T.reshape(-1))
        cols = _local_cols(c)
        for st in range(1, 5):
            idx = cols[st * BLK:(st + 1) * BLK]
            pos[idx] += cp[2 * (st - 1)]
            alls[idx] += cp[2 * (st - 1) + 1]
    valid = pos > 0
    per = np.log(alls + EPS) - np.log(np.where(valid, pos, 1.0))
    nv = int(valid.sum())
    loss = np.where(valid, per, 0.0).sum() / max(nv, 1) if nv > 0 else 0.0
    return np.array(loss, dtype=np.float32)


def kernel(embeddings, labels):
    nc = _get_nc()
    in_maps = make_in_maps(embeddings, labels)
    res = run_bass_kernel_spmd(nc, in_maps, core_ids=list(range(N_CORES)))
    return combine_partials(res.results)


# revision 27
# speedup vs baseline: 1.5721x; 1.0281x over previous
"""MultiLabelContrastiveLoss Trainium2 kernel (8 NeuronCores, Bass/Tile).

Math (reference):
    sim = (emb @ emb.T) / T                      # [B, B]
    cnt[i,j] = #aspects where labels match       # via one-hot GEMM
    positive_mask = (cnt/A >= 0.5) & offdiag
    pos_i = sum_j exp(sim) * positive_mask
    all_i = sum_j exp(sim) * offdiag
    valid = pos > 0
    loss = sum(valid * -log(where(valid,pos,1)/(all+eps))) / max(n_valid, 1)

Kernel strategy (SPMD over 8 cores): exploit sim/mask SYMMETRY — compute
each unordered block-pair once, harvesting BOTH sides of every pair:
  - row side: free-axis accumulation (ACT accum_out for `all`, DVE
    masked-STT accum_out for `pos`),
  - col side: PE ones-matmul column sums of the E and masked-E (junk)
    tiles (accumulated in PSUM across the super-tile, two vectors
    col-packed at output partitions 0/32).
Per core (own row-block = core id c; local column space of 5120 cols):
  L0 = block c  (full 1024, diag excluded via -1e30*I PSUM accumulate,
                 row sums only — both pair orders are inside the block)
  L1..L3 = blocks c+1..c+3 (full, row sums + col sums)
  L4 = block c+4: quadrant split against core c+4 — row-tiles 0-3
       process local cols [0,512), 4-7 [512,1024); the host rotates the
       block's columns by 512 for cores >= 4 so the two cores cover
       complementary quadrants of the pair block.
Work: 36 eq-tiles/core vs 64 for the row-stripe baseline.
The cross-core reduction of the per-row partial sums plus the O(B) log
epilogue happens in the host-side combine (unshard) step: on-device
collectives cost 40-80us fixed under this runtime (measured via
ReduceScatter/AllGather microbenchmarks) — unusable at this kernel size.
"""

import numpy as np
import ml_dtypes

import concourse.tile as tile
from concourse import bacc, mybir
from concourse.bass_utils import run_bass_kernel_spmd

F32 = mybir.dt.float32
BF16 = mybir.dt.bfloat16

B = 8192          # batch
D = 128           # embedding dim
A = 12            # aspects
NCLS = 3          # classes
TEMP = 0.07
EPS = 1e-8
NEG = -1.0e30
N_CORES = 8
BLK = B // N_CORES            # 1024 rows per block
W = 5 * BLK                   # local column space (L4 uses 512/row-tile)
# one-hot GEMM layout: class c's 12 rows live at partitions [32c, 32c+12)
# (engine write base partitions must be 32-aligned), augment row at
# partition 96, gap rows zeroed -> contraction K = 97.
KOH = 97
NST = 5                       # super-tiles per core
NSLOT = 2 * 8 * NST           # pos accum slots (2 cnt halves per tile)


def build(n_cores=N_CORES):
    nc = bacc.Bacc("TRN2", target_bir_lowering=False, debug=False,
                   num_devices=n_cores)
    embT_d = nc.dram_tensor("embT", [D, W], BF16, kind="ExternalInput")
    # labels pre-shifted per class on host: row 32c+a = label[a]-c, filler -1
    labT_d = nc.dram_tensor("labT", [96, W], BF16, kind="ExternalInput")
    # augment rows (row0: -5.5 for ohR, row1: 1.0 for ohL)
    aug_d = nc.dram_tensor("aug", [2, W], BF16, kind="ExternalInput")
    ident_d = nc.dram_tensor("identb", [128, 128], BF16, kind="ExternalInput")
    dneg_d = nc.dram_tensor("diagnegb", [128, 128], BF16, kind="ExternalInput")
    rowpart_d = nc.dram_tensor("rowpart", [128, 16], F32,
                               kind="ExternalOutput")
    colpart_d = nc.dram_tensor("colpart", [8, BLK], F32,
                               kind="ExternalOutput")

    with tile.TileContext(nc) as tc:
        with (
            tc.tile_pool(name="const", bufs=1) as cpool,
            tc.tile_pool(name="ework", bufs=12) as epool,
            tc.tile_pool(name="junk", bufs=12) as jpool,
            tc.tile_pool(name="psA", bufs=2, space="PSUM") as psA,
            tc.tile_pool(name="psB", bufs=2, space="PSUM") as psB,
            tc.tile_pool(name="psC", bufs=1, space="PSUM") as psC,
        ):
            emb_sb = cpool.tile([D, W], BF16)
            lab_sb = cpool.tile([96, W], BF16)
            ident_sb = cpool.tile([128, 128], BF16)
            dneg_sb = cpool.tile([128, 128], BF16)
            ohL = cpool.tile([KOH, BLK], BF16)
            ohR = cpool.tile([KOH, W], BF16)
            ones_sb = cpool.tile([128, 1], BF16)
            pos_acc = cpool.tile([128, NSLOT], F32)
            all_acc = cpool.tile([128, 8 * NST], F32)

            # ---- input DMAs, spread across the DMA-capable queues so the
            # first tiles' operands land fast ----
            nc.sync.dma_start(emb_sb[:, 0:512], embT_d[:, 0:512])
            nc.gpsimd.dma_start(emb_sb[:, 512:1024], embT_d[:, 512:1024])
            nc.sync.dma_start(ident_sb[:], ident_d[:])
            nc.sync.dma_start(dneg_sb[:], dneg_d[:])
            nc.scalar.dma_start(lab_sb[:, 0:BLK], labT_d[:, 0:BLK])
            nc.scalar.dma_start(ohR[96:97, :], aug_d[0:1, :])
            nc.scalar.dma_start(ohL[96:97, :], aug_d[1:2, 0:BLK])
            qs = [nc.sync, nc.gpsimd]
            for k in range(1, W // 1024):
                sl = slice(k * 1024, (k + 1) * 1024)
                qs[(k - 1) % 2].dma_start(emb_sb[:, sl], embT_d[:, sl])
            nc.scalar.dma_start(lab_sb[:, BLK:W], labT_d[:, BLK:W])

            # ---- one-hot build (bf16, 4x mode on DVE) ----
            nc.vector.tensor_scalar(
                out=ohR[0:96, 0:BLK], in0=lab_sb[:, 0:BLK],
                scalar1=0.0, scalar2=None, op0=mybir.AluOpType.is_equal)
            nc.vector.tensor_scalar(
                out=ohL[0:96, :], in0=lab_sb[:, 0:BLK],
                scalar1=0.0, scalar2=None, op0=mybir.AluOpType.is_equal)
            nc.vector.tensor_scalar(
                out=ohR[0:96, BLK:W], in0=lab_sb[:, BLK:W],
                scalar1=0.0, scalar2=None, op0=mybir.AluOpType.is_equal)

            nc.vector.memset(ones_sb[:], 1.0)
            nc.vector.memset(pos_acc[:], 0.0)

            # E / junk tiles per super-tile (kept for the colsum bursts)
            etiles = [[None] * 8 for _ in range(NST)]
            jtiles = [[None] * 8 for _ in range(NST)]

            def tile_cols(st, rl):
                """Local column range of tile (st, rl) and its width."""
                if st < 4:
                    return st * 1024, 1024
                return 4 * 1024 + (0 if rl < 4 else 512), 512

            def emit_mask_stage(pend):
                """cnt GEMM halves + masked STT for a finished sim tile."""
                st, rl, e_t = pend
                c0, w = tile_cols(st, rl)
                rsl = slice(128 * rl, 128 * rl + 128)
                junk = jpool.tile([128, 1024], BF16, tag="junk",
                                  name=f"junk{st}_{rl}")
                jtiles[st][rl] = junk
                for h in range(w // 512):
                    cnt_ps = psB.tile([128, 512], F32, tag="cnt",
                                      name=f"cnt{st}_{rl}_{h}")
                    csl = slice(c0 + 512 * h, c0 + 512 * (h + 1))
                    osl = slice(512 * h, 512 * (h + 1))
                    nc.tensor.matmul(cnt_ps[:], ohL[:, rsl], ohR[:, csl],
                                     start=True, stop=True)
                    slot = (rl * NST + st) * 2 + h
                    nc.vector.scalar_tensor_tensor(
                        out=junk[:, osl],
                        in0=cnt_ps[:], scalar=0.0, in1=e_t[:, osl],
                        op0=mybir.AluOpType.is_ge,
                        op1=mybir.AluOpType.mult,
                        accum_out=pos_acc[:, slot:slot + 1],
                    )

            def emit_colsum_burst(st):
                """Column sums of junk (pos, psum partition 0) and E (all,
                partition 32) tiles of super-tile st >= 1: ones-matmuls
                PE-accumulated across the 8 row-tiles, then evacuated."""
                colps = psC.tile([33, 1024], F32, tag="col", name=f"col{st}")
                for rl in range(8):
                    c0, w = tile_cols(st, rl)
                    if st == 4:
                        start, stop = rl % 4 == 0, rl % 4 == 3
                    else:
                        start, stop = rl == 0, rl == 7
                    for hh in range(w // 512):
                        o0 = (c0 - 4096) if st == 4 else 512 * hh
                        osl = slice(o0, o0 + 512)
                        isl = slice(512 * hh, 512 * hh + 512)
                        for v, tiles in enumerate((jtiles, etiles)):
                            vp = 32 * v
                            nc.tensor.matmul(
                                colps[vp:vp + 1, osl], ones_sb[:],
                                tiles[st][rl][:, isl],
                                start=start, stop=stop,
                                tile_position=(0, vp),
                            )
                # PSUM is not DMA-readable: one [33,1024] copy (covers both
                # vectors' partitions) on ACT, which has more slack than DVE.
                colsb = cpool.tile([33, 1024], F32, name=f"colsb{st}")
                if st % 2 == 0:
                    nc.vector.tensor_copy(colsb[:], colps[:])
                else:
                    nc.scalar.copy(colsb[:], colps[:])
                for v in range(2):
                    nc.sync.dma_start(
                        colpart_d[2 * (st - 1) + v:2 * (st - 1) + v + 1, :],
                        colsb[32 * v:32 * v + 1, :])

            # ---- main loop (software-pipelined: the mask stage of tile
            # k-1 is emitted after sim+exp of tile k so the PE's in-order
            # queue never makes ACT wait behind DVE-gated cnt matmuls;
            # colsum burst of super-tile st-1 is emitted inside (st, rl==2))
            pending = None
            for st in range(NST):
                for rl in range(8):
                    c0, w = tile_cols(st, rl)
                    rsl = slice(128 * rl, 128 * rl + 128)
                    sim_ps = psA.tile([128, 1024], F32, tag="sim",
                                      name=f"sim{st}_{rl}")
                    dloc = 128 * rl  # diag offset within L0 (local cols)
                    for h in range(w // 512):
                        csl = slice(c0 + 512 * h, c0 + 512 * (h + 1))
                        osl = slice(512 * h, 512 * (h + 1))
                        dh = st == 0 and 512 * h <= dloc < 512 * (h + 1)
                        nc.tensor.matmul(sim_ps[:, osl], emb_sb[:, rsl],
                                         emb_sb[:, csl], start=True,
                                         stop=not dh)
                        if dh:
                            # exact diag exclusion: accumulate -1e30*I
                            dsl = slice(dloc, dloc + 128)
                            nc.tensor.matmul(sim_ps[:, dsl], ident_sb[:],
                                             dneg_sb[:], start=False,
                                             stop=True)
                    e_t = epool.tile([128, 1024], BF16, tag="E",
                                     name=f"E{st}_{rl}")
                    etiles[st][rl] = e_t
                    aslot = rl * NST + st
                    nc.scalar.activation(
                        e_t[:, 0:w], sim_ps[:, 0:w],
                        mybir.ActivationFunctionType.Exp,
                        scale=1.0 / TEMP,
                        accum_out=all_acc[:, aslot:aslot + 1],
                    )
                    if pending is not None:
                        emit_mask_stage(pending)
                    pending = (st, rl, e_t)
                    if rl == 2 and st >= 2:
                        emit_colsum_burst(st - 1)
            emit_mask_stage(pending)
            emit_colsum_burst(4)

            # ---- row-partial reduction -> [128, 16] out ----
            pr_pack = cpool.tile([128, 16], F32)
            nc.vector.reduce_sum(
                pr_pack[:, 0:8],
                pos_acc[:].rearrange("p (r q) -> p r q", q=2 * NST),
                axis=mybir.AxisListType.X)
            nc.vector.reduce_sum(
                pr_pack[:, 8:16],
                all_acc[:].rearrange("p (r q) -> p r q", q=NST),
                axis=mybir.AxisListType.X)
            nc.sync.dma_start(rowpart_d[:], pr_pack[:])

    nc.compile()
    return nc


_CACHE = {}


def _get_nc():
    if "nc" not in _CACHE:
        _CACHE["nc"] = build()
    return _CACHE["nc"]


def _local_cols(c):
    """Global column indices of core c's local [0, W) column space."""
    cols = []
    for s in range(4):
        b = (c + s) % N_CORES
        cols.extend(range(b * BLK, (b + 1) * BLK))
    b4 = (c + 4) % N_CORES
    half = np.arange(BLK)
    if c >= 4:
        half = np.roll(half, -512)
    cols.extend(b4 * BLK + half)
    return np.array(cols)


def make_in_maps(embeddings, labels):
    embT = np.ascontiguousarray(
        embeddings.astype(np.float32).T).astype(ml_dtypes.bfloat16)  # [D, B]
    labT = np.full((96, B), -1.0, dtype=np.float32)
    lt = np.asarray(labels).astype(np.float32).T
    for c in range(NCLS):
        labT[32 * c:32 * c + A] = lt - c
    labT = labT.astype(ml_dtypes.bfloat16)
    ident = np.eye(128, dtype=np.float32).astype(ml_dtypes.bfloat16)
    dneg = (np.eye(128, dtype=np.float32) * NEG).astype(ml_dtypes.bfloat16)
    aug = np.empty((2, W), dtype=np.float32)
    aug[0] = -(A / 2.0 - 0.5)
    aug[1] = 1.0
    aug = aug.astype(ml_dtypes.bfloat16)

    in_maps = []
    for c in range(N_CORES):
        cols = _local_cols(c)
        in_maps.append({
            "embT": np.ascontiguousarray(embT[:, cols]),
            "labT": np.ascontiguousarray(labT[:, cols]),
            "aug": aug, "identb": ident, "diagnegb": dneg,
        })
    return in_maps


def combine_partials(results):
    """Cross-core reduce of the partial row/col sums + scalar epilogue
    (the unshard step; O(B) host work)."""
    pos = np.zeros(B, dtype=np.float64)
    alls = np.zeros(B, dtype=np.float64)
    for c in range(N_CORES):
        rp = results[c]["rowpart"].astype(np.float64)     # [128, 16]
        cp = results[c]["colpart"].astype(np.float64)     # [8, BLK]
        rows = c * BLK + (np.arange(8) * 128)[:, None] + np.arange(128)
        np.add.at(pos, rows.reshape(-1), rp[:, 0:8].T.reshape(-1))
        np.add.at(alls, rows.reshape(-1), rp[:, 8:16].T.reshape(-1))
        cols = _local_cols(c)
        for st in range(1, 5):
            idx = cols[st * BLK:(st + 1) * BLK]
            pos[idx] += cp[2 * (st - 1)]
            alls[idx] += cp[2 * (st - 1) + 1]
    valid = pos > 0
    per = np.log(alls + EPS) - np.log(np.where(valid, pos, 1.0))
    nv = int(valid.sum())
    loss = np.where(valid, per, 0.0).sum() / max(nv, 1) if nv > 0 else 0.0
    return np.array(loss, dtype=np.float32)


def kernel(embeddings, labels):
    nc = _get_nc()
    in_maps = make_in_maps(embeddings, labels)
    res = run_bass_kernel_spmd(nc, in_maps, core_ids=list(range(N_CORES)))
    return combine_partials(res.results)


# revision 28
# speedup vs baseline: 1.5933x; 1.0135x over previous
"""MultiLabelContrastiveLoss Trainium2 kernel (8 NeuronCores, Bass/Tile).

Math (reference):
    sim = (emb @ emb.T) / T                      # [B, B]
    cnt[i,j] = #aspects where labels match       # via one-hot GEMM
    positive_mask = (cnt/A >= 0.5) & offdiag
    pos_i = sum_j exp(sim) * positive_mask
    all_i = sum_j exp(sim) * offdiag
    valid = pos > 0
    loss = sum(valid * -log(where(valid,pos,1)/(all+eps))) / max(n_valid, 1)

Kernel strategy (SPMD over 8 cores): exploit sim/mask SYMMETRY — compute
each unordered block-pair once, harvesting BOTH sides of every pair:
  - row side: free-axis accumulation (ACT accum_out for `all`, DVE
    masked-STT accum_out for `pos`),
  - col side: PE ones-matmul column sums of the E and masked-E (junk)
    tiles (accumulated in PSUM across the super-tile, two vectors
    col-packed at output partitions 0/32).
Per core (own row-block = core id c; local column space of 5120 cols):
  L0 = block c  (full 1024, diag excluded via -1e30*I PSUM accumulate,
                 row sums only — both pair orders are inside the block)
  L1..L3 = blocks c+1..c+3 (full, row sums + col sums)
  L4 = block c+4: quadrant split against core c+4 — row-tiles 0-3
       process local cols [0,512), 4-7 [512,1024); the host rotates the
       block's columns by 512 for cores >= 4 so the two cores cover
       complementary quadrants of the pair block.
Work: 36 eq-tiles/core vs 64 for the row-stripe baseline.
The cross-core reduction of the per-row partial sums plus the O(B) log
epilogue happens in the host-side combine (unshard) step: on-device
collectives cost 40-80us fixed under this runtime (measured via
ReduceScatter/AllGather microbenchmarks) — unusable at this kernel size.
"""

import numpy as np
import ml_dtypes

import concourse.tile as tile
from concourse import bacc, mybir
from concourse.bass_utils import run_bass_kernel_spmd

F32 = mybir.dt.float32
BF16 = mybir.dt.bfloat16

B = 8192          # batch
D = 128           # embedding dim
A = 12            # aspects
NCLS = 3          # classes
TEMP = 0.07
EPS = 1e-8
NEG = -1.0e30
N_CORES = 8
BLK = B // N_CORES            # 1024 rows per block
W = 5 * BLK                   # local column space (L4 uses 512/row-tile)
# one-hot GEMM layout: class c's 12 rows live at partitions [32c, 32c+12)
# (engine write base partitions must be 32-aligned), augment row at
# partition 96, gap rows zeroed -> contraction K = 97.
KOH = 97
NST = 5                       # super-tiles per core
NSLOT = 2 * 8 * NST           # pos accum slots (2 cnt halves per tile)


def build(n_cores=N_CORES):
    nc = bacc.Bacc("TRN2", target_bir_lowering=False, debug=False,
                   num_devices=n_cores)
    embT_d = nc.dram_tensor("embT", [D, W], BF16, kind="ExternalInput")
    # labels pre-shifted per class on host: row 32c+a = label[a]-c, filler -1
    labT_d = nc.dram_tensor("labT", [96, W], BF16, kind="ExternalInput")
    # augment rows (row0: -5.5 for ohR, row1: 1.0 for ohL)
    aug_d = nc.dram_tensor("aug", [2, W], BF16, kind="ExternalInput")
    ident_d = nc.dram_tensor("identb", [128, 128], BF16, kind="ExternalInput")
    dneg_d = nc.dram_tensor("diagnegb", [128, 128], BF16, kind="ExternalInput")
    rowpart_d = nc.dram_tensor("rowpart", [128, 16], F32,
                               kind="ExternalOutput")
    colpart_d = nc.dram_tensor("colpart", [8, BLK], F32,
                               kind="ExternalOutput")

    with tile.TileContext(nc) as tc:
        with (
            tc.tile_pool(name="const", bufs=1) as cpool,
            tc.tile_pool(name="ework", bufs=12) as epool,
            tc.tile_pool(name="junk", bufs=12) as jpool,
            tc.tile_pool(name="psA", bufs=2, space="PSUM") as psA,
            tc.tile_pool(name="psB", bufs=2, space="PSUM") as psB,
            tc.tile_pool(name="psC", bufs=1, space="PSUM") as psC,
        ):
            emb_sb = cpool.tile([D, W], BF16)
            lab_sb = cpool.tile([96, W], BF16)
            ident_sb = cpool.tile([128, 128], BF16)
            dneg_sb = cpool.tile([128, 128], BF16)
            ohL = cpool.tile([KOH, BLK], BF16)
            ohR = cpool.tile([KOH, W], BF16)
            ones_sb = cpool.tile([128, 1], BF16)
            pos_acc = cpool.tile([128, NSLOT], F32)
            all_acc = cpool.tile([128, 8 * NST], F32)

            # ---- input DMAs in consumption order ----
            nc.sync.dma_start(emb_sb[:, 0:512], embT_d[:, 0:512])
            nc.sync.dma_start(ident_sb[:], ident_d[:])
            nc.sync.dma_start(dneg_sb[:], dneg_d[:])
            nc.sync.dma_start(emb_sb[:, 512:1024], embT_d[:, 512:1024])
            nc.sync.dma_start(lab_sb[:, 0:BLK], labT_d[:, 0:BLK])
            nc.sync.dma_start(ohR[96:97, :], aug_d[0:1, :])
            nc.sync.dma_start(ohL[96:97, :], aug_d[1:2, 0:BLK])
            for k in range(1, W // 1024):
                sl = slice(k * 1024, (k + 1) * 1024)
                nc.sync.dma_start(emb_sb[:, sl], embT_d[:, sl])
                if k == 1:
                    nc.sync.dma_start(lab_sb[:, BLK:W], labT_d[:, BLK:W])

            # ---- one-hot build (bf16, 4x mode on DVE) ----
            nc.vector.tensor_scalar(
                out=ohR[0:96, 0:BLK], in0=lab_sb[:, 0:BLK],
                scalar1=0.0, scalar2=None, op0=mybir.AluOpType.is_equal)
            nc.vector.tensor_scalar(
                out=ohL[0:96, :], in0=lab_sb[:, 0:BLK],
                scalar1=0.0, scalar2=None, op0=mybir.AluOpType.is_equal)
            nc.vector.tensor_scalar(
                out=ohR[0:96, BLK:W], in0=lab_sb[:, BLK:W],
                scalar1=0.0, scalar2=None, op0=mybir.AluOpType.is_equal)

            nc.vector.memset(ones_sb[:], 1.0)
            nc.vector.memset(pos_acc[:], 0.0)

            # E / junk tiles per super-tile (kept for the colsum bursts)
            etiles = [[None] * 8 for _ in range(NST)]
            jtiles = [[None] * 8 for _ in range(NST)]

            def tile_cols(st, rl):
                """Local column range of tile (st, rl) and its width."""
                if st < 4:
                    return st * 1024, 1024
                return 4 * 1024 + (0 if rl < 4 else 512), 512

            def emit_mask_stage(pend):
                """cnt GEMM halves + masked STT for a finished sim tile."""
                st, rl, e_t = pend
                c0, w = tile_cols(st, rl)
                rsl = slice(128 * rl, 128 * rl + 128)
                junk = jpool.tile([128, 1024], BF16, tag="junk",
                                  name=f"junk{st}_{rl}")
                jtiles[st][rl] = junk
                for h in range(w // 512):
                    cnt_ps = psB.tile([128, 512], F32, tag="cnt",
                                      name=f"cnt{st}_{rl}_{h}")
                    csl = slice(c0 + 512 * h, c0 + 512 * (h + 1))
                    osl = slice(512 * h, 512 * (h + 1))
                    nc.tensor.matmul(cnt_ps[:], ohL[:, rsl], ohR[:, csl],
                                     start=True, stop=True)
                    slot = (rl * NST + st) * 2 + h
                    nc.vector.scalar_tensor_tensor(
                        out=junk[:, osl],
                        in0=cnt_ps[:], scalar=0.0, in1=e_t[:, osl],
                        op0=mybir.AluOpType.is_ge,
                        op1=mybir.AluOpType.mult,
                        accum_out=pos_acc[:, slot:slot + 1],
                    )

            def emit_colsum_burst(st):
                """Column sums of junk (pos, psum partition 0) and E (all,
                partition 32) tiles of super-tile st >= 1: ones-matmuls
                PE-accumulated across the 8 row-tiles, then evacuated."""
                colps = psC.tile([33, 1024], F32, tag="col", name=f"col{st}")
                for rl in range(8):
                    c0, w = tile_cols(st, rl)
                    if st == 4:
                        start, stop = rl % 4 == 0, rl % 4 == 3
                    else:
                        start, stop = rl == 0, rl == 7
                    for hh in range(w // 512):
                        o0 = (c0 - 4096) if st == 4 else 512 * hh
                        osl = slice(o0, o0 + 512)
                        isl = slice(512 * hh, 512 * hh + 512)
                        for v, tiles in enumerate((jtiles, etiles)):
                            vp = 32 * v
                            nc.tensor.matmul(
                                colps[vp:vp + 1, osl], ones_sb[:],
                                tiles[st][rl][:, isl],
                                start=start, stop=stop,
                                tile_position=(0, vp),
                            )
                # PSUM is not DMA-readable: one [33,1024] copy (covers both
                # vectors' partitions) on ACT, which has more slack than DVE.
                colsb = cpool.tile([33, 1024], F32, name=f"colsb{st}")
                if st % 2 == 0:
                    nc.vector.tensor_copy(colsb[:], colps[:])
                else:
                    nc.scalar.copy(colsb[:], colps[:])
                for v in range(2):
                    nc.sync.dma_start(
                        colpart_d[2 * (st - 1) + v:2 * (st - 1) + v + 1, :],
                        colsb[32 * v:32 * v + 1, :])

            # ---- main loop (software-pipelined: the mask stage of tile
            # k-1 is emitted after sim+exp of tile k so the PE's in-order
            # queue never makes ACT wait behind DVE-gated cnt matmuls;
            # colsum burst of super-tile st-1 is emitted inside (st, rl==2))
            pending = None
            for st in range(NST):
                for rl in range(8):
                    c0, w = tile_cols(st, rl)
                    rsl = slice(128 * rl, 128 * rl + 128)
                    sim_ps = psA.tile([128, 1024], F32, tag="sim",
                                      name=f"sim{st}_{rl}")
                    dloc = 128 * rl  # diag offset within L0 (local cols)
                    for h in range(w // 512):
                        csl = slice(c0 + 512 * h, c0 + 512 * (h + 1))
                        osl = slice(512 * h, 512 * (h + 1))
                        dh = st == 0 and 512 * h <= dloc < 512 * (h + 1)
                        nc.tensor.matmul(sim_ps[:, osl], emb_sb[:, rsl],
                                         emb_sb[:, csl], start=True,
                                         stop=not dh)
                        if dh:
                            # exact diag exclusion: accumulate -1e30*I
                            dsl = slice(dloc, dloc + 128)
                            nc.tensor.matmul(sim_ps[:, dsl], ident_sb[:],
                                             dneg_sb[:], start=False,
                                             stop=True)
                    e_t = epool.tile([128, 1024], BF16, tag="E",
                                     name=f"E{st}_{rl}")
                    etiles[st][rl] = e_t
                    aslot = rl * NST + st
                    nc.scalar.activation(
                        e_t[:, 0:w], sim_ps[:, 0:w],
                        mybir.ActivationFunctionType.Exp,
                        scale=1.0 / TEMP,
                        accum_out=all_acc[:, aslot:aslot + 1],
                    )
                    if pending is not None:
                        emit_mask_stage(pending)
                    pending = (st, rl, e_t)
                    if rl == 2 and st >= 2:
                        emit_colsum_burst(st - 1)
            emit_mask_stage(pending)
            emit_colsum_burst(4)

            # ---- row-partial reduction -> [128, 16] out ----
            pr_pack = cpool.tile([128, 16], F32)
            nc.vector.reduce_sum(
                pr_pack[:, 0:8],
                pos_acc[:].rearrange("p (r q) -> p r q", q=2 * NST),
                axis=mybir.AxisListType.X)
            nc.vector.reduce_sum(
                pr_pack[:, 8:16],
                all_acc[:].rearrange("p (r q) -> p r q", q=NST),
                axis=mybir.AxisListType.X)
            nc.sync.dma_start(rowpart_d[:], pr_pack[:])

    nc.compile()
    return nc


_CACHE = {}


def _get_nc():
    if "nc" not in _CACHE:
        _CACHE["nc"] = build()
    return _CACHE["nc"]


def _local_cols(c):
    """Global column indices of core c's local [0, W) column space."""
    cols = []
    for s in range(4):
        b = (c + s) % N_CORES
        cols.extend(range(b * BLK, (b + 1) * BLK))
    b4 = (c + 4) % N_CORES
    half = np.arange(BLK)
    if c >= 4:
        half = np.roll(half, -512)
    cols.extend(b4 * BLK + half)
    return np.array(cols)


def make_in_maps(embeddings, labels):
    embT = np.ascontiguousarray(
        embeddings.astype(np.float32).T).astype(ml_dtypes.bfloat16)  # [D, B]
    labT = np.full((96, B), -1.0, dtype=np.float32)
    lt = np.asarray(labels).astype(np.float32).T
    for c in range(NCLS):
        labT[32 * c:32 * c + A] = lt - c
    labT = labT.astype(ml_dtypes.bfloat16)
    ident = np.eye(128, dtype=np.float32).astype(ml_dtypes.bfloat16)
    dneg = (np.eye(128, dtype=np.float32) * NEG).astype(ml_dtypes.bfloat16)
    aug = np.empty((2, W), dtype=np.float32)
    aug[0] = -(A / 2.0 - 0.5)
    aug[1] = 1.0
    aug = aug.astype(ml_dtypes.bfloat16)

    in_maps = []
    for c in range(N_CORES):
        cols = _local_cols(c)
        in_maps.append({
            "embT": np.ascontiguousarray(embT[:, cols]),
            "labT": np.ascontiguousarray(labT[:, cols]),
            "aug": aug, "identb": ident, "diagnegb": dneg,
        })
    return in_maps


def combine_partials(results):
    """Cross-core reduce of the partial row/col sums + scalar epilogue
    (the unshard step; O(B) host work)."""
    pos = np.zeros(B, dtype=np.float64)
    alls = np.zeros(B, dtype=np.float64)
    for c in range(N_CORES):
        rp = results[c]["rowpart"].astype(np.float64)     # [128, 16]
        cp = results[c]["colpart"].astype(np.float64)     # [8, BLK]
        rows = c * BLK + (np.arange(8) * 128)[:, None] + np.arange(128)
        np.add.at(pos, rows.reshape(-1), rp[:, 0:8].T.reshape(-1))
        np.add.at(alls, rows.reshape(-1), rp[:, 8:16].T.reshape(-1))
        cols = _local_cols(c)
        for st in range(1, 5):
            idx = cols[st * BLK:(st + 1) * BLK]
            pos[idx] += cp[2 * (st - 1)]
            alls[idx] += cp[2 * (st - 1) + 1]
    valid = pos > 0
    per = np.log(alls + EPS) - np.log(np.where(valid, pos, 1.0))
    nv = int(valid.sum())
    loss = np.where(valid, per, 0.0).sum() / max(nv, 1) if nv > 0 else 0.0
    return np.array(loss, dtype=np.float32)


def kernel(embeddings, labels):
    nc = _get_nc()
    in_maps = make_in_maps(embeddings, labels)
    res = run_bass_kernel_spmd(nc, in_maps, core_ids=list(range(N_CORES)))
    return combine_partials(res.results)
